# revision 15
# baseline (speedup 1.0000x reference)
"""Trainium2 Bass kernel for FlowNet-C CorrelationCost.

Problem: out[b,i,j, tj*21+ti] = (1/C) * sum_c A[b,i,j,c] * Bz[b, i+dy, j+dx, c]
with dy = 2*tj - 20, dx = 2*ti - 20, Bz = B zero-padded by 20 spatially.
Shapes: A, B = [16, 48, 64, 256] f32 -> out [16, 48, 64, 441] f32.

Strategy (v2)
-------------
- Pure data-parallel: batch 16 -> 2 images per NeuronCore (8 cores, SPMD).
- Host pre-transposes to channel-major, prescales by 1/16 (so the two input
  scales bake the exact 1/256 output scale) and rounds to fp16. Tolerance is
  2e-2; fp16 inputs give ~1.5e-4, so no hi/lo split -> only 2 K-passes of
  128 channels each (3x less PE work than the fp32-exact hi/lo scheme).
- PE formulation: stationary = A[c, 4 same-parity rows x 32 same-parity
  cols] (128x128); moving = B[c, r x 32 same-parity cols] for all B rows r
  within +-20 of the pack. PSUM[(i,j), (r,jj)] holds every correlation
  product with dy = r - i, dx = jj - j.
- Per (batch, group, col-parity): one 2-bank PSUM supertile holds both
  r-chunks; the 2 matmul passes accumulate fp32; ONE copy (alternating
  VectorE / ScalarE) casts PSUM -> fp16 SBUF; ONE DMA (alternating SP ring /
  Pool ring) ships the compact block. Output bytes are halved vs fp32.
- Input DMAs: batch 0 arrives as fine-grained quarters on the SP ring (PE
  unblocks ~2.5us in); batch 1 as big per-(b,chunk) tiles on the Pool ring.

The harness calls kernel(**inputs) with the FULL inputs; this file is
self-contained (shapes hardcoded).
"""

from contextlib import ExitStack

import numpy as np

import concourse.bass as bass
import concourse.tile as tile
from concourse import bacc, mybir

B_FULL, H, W, C = 16, 48, 64, 256
N_CORES = 8
B_PER = B_FULL // N_CORES  # batches per core
MD = 20                    # max displacement
D = 21                     # displacements per axis
PACK = 4                   # i rows packed into one stationary operand
F32 = mybir.dt.float32
F16 = mybir.dt.float16
N_CC = C // 128            # channel chunks (K-passes)
OUT_W = 704                # max per-(b,gi,p) out cols (2 chunks x 11 r x 32)


def plan_groups():
    """(pack, r_list) per i-pack: pack = 4 same-parity rows, r_list = B rows
    (same parity, step 2) needed by any row in the pack."""
    groups = []
    for par in (0, 1):
        i_vals = list(range(par, H, 2))
        for k in range(0, len(i_vals), PACK):
            pack = i_vals[k:k + PACK]
            r_lo = max(0, pack[0] - MD)
            r_hi = min(H - 1, pack[-1] + MD)
            r_list = [r for r in range(r_lo, r_hi + 1) if (r - pack[0]) % 2 == 0]
            groups.append((pack, r_list))
    return groups


def chunk_rs(r_list):
    """Split the r list into chunks of <= 16 rows (<= 512 cols, one PSUM
    bank)."""
    n = len(r_list)
    if n <= 16:
        return [r_list]
    h = (n + 1) // 2
    return [r_list[:h], r_list[h:]]


GROUPS = plan_groups()
N_GROUPS = len(GROUPS)  # 12 i-packs


def prep_inputs(input_a, input_b):
    """Full [B, H, W, C] f32 inputs -> packed fp16 device arrays.

    a_packed[b, c, par, pk, p, k*32+j32] = a[b, 8*pk+2*k+par, 2*j32+p, c]/16
    b_packed[b, c, p, par, r2*32+jj32]  = b[b, 2*r2+par, 2*jj32+p, c]/16
    """
    s = np.float16(1.0)  # applied after f32 * (1/16)
    at = (np.asarray(input_a, np.float32) * np.float32(1 / 16)).astype(np.float16)
    bt = (np.asarray(input_b, np.float32) * np.float32(1 / 16)).astype(np.float16)
    at = at.transpose(0, 3, 1, 2)  # [B, C, H, W]
    bt = bt.transpose(0, 3, 1, 2)
    nb = at.shape[0]
    ap = at.reshape(nb, C, 6, PACK, 2, 32, 2).transpose(0, 1, 4, 2, 6, 3, 5)
    bp = bt.reshape(nb, C, 24, 2, 32, 2).transpose(0, 1, 5, 3, 2, 4)
    del s
    return (np.ascontiguousarray(ap).reshape(nb, C, 2, 6, 2, PACK * 32),
            np.ascontiguousarray(bp).reshape(nb, C, 2, 2, 24 * 32))


def build_program():
    nc = bacc.Bacc("TRN2", target_bir_lowering=False, debug=False)

    a_d = nc.dram_tensor("a_t", [B_PER, C, 2, 6, 2, PACK * 32], F16,
                         kind="ExternalInput")
    b_d = nc.dram_tensor("b_t", [B_PER, C, 2, 2, 24 * 32], F16,
                         kind="ExternalInput")
    o_d = nc.dram_tensor("out_raw", [B_PER, N_GROUPS, 2, 128, OUT_W], F16,
                         kind="ExternalOutput")

    with tile.TileContext(nc) as tc, ExitStack() as ctx:
        inp = ctx.enter_context(tc.tile_pool(name="inp", bufs=1))
        psum = ctx.enter_context(
            tc.tile_pool(name="psum", bufs=4, space=bass.MemorySpace.PSUM))
        stage = ctx.enter_context(tc.tile_pool(name="stage", bufs=8))

        a_sb = {}
        b_sb = {}

        # Inputs stream as fine-grained quarters in exact PE consumption
        # order, ALTERNATING between the SP and ACT rings: the 16-engine DMA
        # pool round-robins across active queues, so two queues double the
        # config (descriptor-gen) rate while each queue's internal order
        # stays monotone in need-time — CoreSim's independent-queue model
        # and the HW pool then agree on arrival order.
        qi = 0
        for b in range(B_PER):
            for par in (0, 1):
                for cc in (0, 1):
                    cs = slice(cc * 128, (cc + 1) * 128)
                    ta = inp.tile([128, 6, 2, PACK * 32], F16,
                                  tag=f"aq{b}{cc}_{par}")
                    eng = nc.sync if qi % 2 == 0 else nc.gpsimd
                    eng.dma_start(ta[:], a_d[b, cs, par])
                    qi += 1
                    a_sb[b, cc, par] = ta
                    for p in (0, 1):
                        tb = inp.tile([128, 24 * 32], F16,
                                      tag=f"bq{b}{cc}_{p}{par}")
                        eng = nc.sync if qi % 2 == 0 else nc.gpsimd
                        eng.dma_start(tb[:], b_d[b, cs, p, par])
                        qi += 1
                        b_sb[b, cc, p, par] = tb

        def lhs_ap(b, cc, par, pk, p):
            return a_sb[b, cc, par][:, pk, p, :]

        def rhs_ap(b, cc, p, par, lo, hi):
            return b_sb[b, cc, p, par][:, lo:hi]

        # One unit per (b, gi, p): both r-chunks land in one 2-bank PSUM
        # supertile, drained by ONE copy and ONE DMA.
        t = 0
        for b in range(B_PER):
            for gi, (pack, r_list) in enumerate(GROUPS):
                par = pack[0] % 2
                pk = (pack[0] // 2) // PACK
                chunks = chunk_rs(r_list)
                nch = len(chunks)
                nr = len(chunks[0])
                ncols = nr * 32
                tot = nch * ncols
                for p in (0, 1):
                    ps = psum.tile([128, 1024], F32, tag="ps")
                    for cc in (0, 1):
                        for ci, rs in enumerate(chunks):
                            r2lo = rs[0] // 2
                            rhs = rhs_ap(b, cc, p, par,
                                         r2lo * 32, (r2lo + nr) * 32)
                            nc.tensor.matmul(
                                ps[:, ci * 512: ci * 512 + ncols],
                                lhs_ap(b, cc, par, pk, p), rhs,
                                start=(cc == 0), stop=(cc == 1),
                            )
                    st = stage.tile([128, OUT_W], F16, tag="st")
                    ps3 = ps.rearrange("q (c n) -> q c n", c=2)
                    st3 = st[:, :tot].rearrange("q (c n) -> q c n", c=nch)
                    if t % 2 == 0:
                        nc.vector.tensor_copy(st3, ps3[:, :nch, :ncols])
                    else:
                        nc.scalar.copy(st3, ps3[:, :nch, :ncols])
                    # outs alternate SP/Pool rings (both free once their
                    # input quarters are configured).
                    eng = nc.sync if t % 2 == 0 else nc.gpsimd
                    eng.dma_start(o_d[b, gi, p][:, :tot], st[:, :tot])
                    t += 1

    nc.compile()
    return nc


_NC_CACHE = None


def _get_program():
    global _NC_CACHE
    if _NC_CACHE is None:
        _NC_CACHE = build_program()
    return _NC_CACHE


def assemble_output(raw_all):
    """raw_all: [nb, N_GROUPS, 2, 128, OUT_W] fp16 (scale already baked)
    -> out [nb, H, W, D*D] f32."""
    nb = raw_all.shape[0]
    raw_all = np.asarray(raw_all, np.float32)
    # band tensor: [nb, H, 2(p), 32(j32), D(dy), 32(jj32)]
    band = np.zeros((nb, H, 2, 32, D, 32), np.float32)
    for gi, (pack, r_list) in enumerate(GROUPS):
        chunks = chunk_rs(r_list)
        nr = len(chunks[0])
        for ci, rs in enumerate(chunks):
            # [nb, 2p, 128, nr*32] -> [nb, 2p, 4k, 32j, nr, 32jj]
            blk = raw_all[:, gi, :, :, ci * nr * 32:(ci + 1) * nr * 32]
            blk = blk.reshape(nb, 2, PACK, 32, nr, 32)
            for k, i in enumerate(pack):
                for ridx, r in enumerate(rs):
                    dy = r - i
                    if abs(dy) > MD:
                        continue
                    dyi = (dy + MD) // 2
                    band[:, i, :, :, dyi, :] = blk[:, :, k, :, ridx, :]
    out = np.zeros((nb, H, W, D, D), np.float32)
    s = band.strides
    for p in (0, 1):
        for ti in range(D):
            delta = ti - MD // 2  # dx/2
            j32_lo = max(0, -delta)
            j32_hi = min(32, 32 - delta)
            n = j32_hi - j32_lo
            if n <= 0:
                continue
            v = np.lib.stride_tricks.as_strided(
                band[:, :, p, j32_lo:, :, j32_lo + delta:],
                shape=(nb, H, n, D),
                strides=(s[0], s[1], s[3] + s[5], s[4]),
            )
            out[:, :, 2 * np.arange(j32_lo, j32_hi) + p, :, ti] = \
                v.transpose(2, 0, 1, 3)
    return out.reshape(nb, H, W, D * D)


def kernel(input_a: np.ndarray, input_b: np.ndarray) -> np.ndarray:
    from concourse.bass_utils import run_bass_kernel_spmd

    a, bt = prep_inputs(input_a, input_b)
    nc = _get_program()
    core_ids = list(range(N_CORES))
    in_maps = [
        {"a_t": a[c * B_PER:(c + 1) * B_PER], "b_t": bt[c * B_PER:(c + 1) * B_PER]}
        for c in core_ids
    ]
    res = run_bass_kernel_spmd(nc, in_maps, core_ids)
    raw_all = np.concatenate(
        [res.results[c]["out_raw"] for c in core_ids], axis=0)
    return assemble_output(raw_all)


# revision 17
# speedup vs baseline: 1.0762x; 1.0762x over previous
"""Trainium2 Bass kernel for FlowNet-C CorrelationCost.

Problem: out[b,i,j, tj*21+ti] = (1/C) * sum_c A[b,i,j,c] * Bz[b, i+dy, j+dx, c]
with dy = 2*tj - 20, dx = 2*ti - 20, Bz = B zero-padded by 20 spatially.
Shapes: A, B = [16, 48, 64, 256] f32 -> out [16, 48, 64, 441] f32.

Strategy (v2)
-------------
- Pure data-parallel: batch 16 -> 2 images per NeuronCore (8 cores, SPMD).
- Host pre-transposes to channel-major, prescales by 1/16 (so the two input
  scales bake the exact 1/256 output scale) and rounds to fp16. Tolerance is
  2e-2; fp16 inputs give ~1.5e-4, so no hi/lo split -> only 2 K-passes of
  128 channels each (3x less PE work than the fp32-exact hi/lo scheme).
- PE formulation: stationary = A[c, 4 same-parity rows x 32 same-parity
  cols] (128x128); moving = B[c, r x 32 same-parity cols] for all B rows r
  within +-20 of the pack. PSUM[(i,j), (r,jj)] holds every correlation
  product with dy = r - i, dx = jj - j.
- Per (batch, group, col-parity): one 2-bank PSUM supertile holds both
  r-chunks; the 2 matmul passes accumulate fp32; ONE copy (alternating
  VectorE / ScalarE) casts PSUM -> fp16 SBUF; ONE DMA (alternating SP ring /
  Pool ring) ships the compact block. Output bytes are halved vs fp32.
- Input DMAs: batch 0 arrives as fine-grained quarters on the SP ring (PE
  unblocks ~2.5us in); batch 1 as big per-(b,chunk) tiles on the Pool ring.

The harness calls kernel(**inputs) with the FULL inputs; this file is
self-contained (shapes hardcoded).
"""

from contextlib import ExitStack

import numpy as np

import concourse.bass as bass
import concourse.tile as tile
from concourse import bacc, mybir

B_FULL, H, W, C = 16, 48, 64, 256
N_CORES = 8
B_PER = B_FULL // N_CORES  # batches per core
MD = 20                    # max displacement
D = 21                     # displacements per axis
PACK = 4                   # i rows packed into one stationary operand
F32 = mybir.dt.float32
F16 = mybir.dt.float16
N_CC = C // 128            # channel chunks (K-passes)
OUT_W = 704                # max per-(b,gi,p) out cols (2 chunks x 11 r x 32)


def plan_groups():
    """(pack, r_list) per i-pack: pack = 4 same-parity rows, r_list = B rows
    (same parity, step 2) needed by any row in the pack."""
    groups = []
    for par in (0, 1):
        i_vals = list(range(par, H, 2))
        for k in range(0, len(i_vals), PACK):
            pack = i_vals[k:k + PACK]
            r_lo = max(0, pack[0] - MD)
            r_hi = min(H - 1, pack[-1] + MD)
            r_list = [r for r in range(r_lo, r_hi + 1) if (r - pack[0]) % 2 == 0]
            groups.append((pack, r_list))
    return groups


def chunk_rs(r_list):
    """Split the r list into chunks of <= 16 rows (<= 512 cols, one PSUM
    bank)."""
    n = len(r_list)
    if n <= 16:
        return [r_list]
    h = (n + 1) // 2
    return [r_list[:h], r_list[h:]]


GROUPS = plan_groups()
N_GROUPS = len(GROUPS)  # 12 i-packs


def prep_inputs(input_a, input_b):
    """Full [B, H, W, C] f32 inputs -> packed fp16 device arrays.

    a_packed[b, c, par, pk, p, k*32+j32] = a[b, 8*pk+2*k+par, 2*j32+p, c]/16
    b_packed[b, c, p, par, r2*32+jj32]  = b[b, 2*r2+par, 2*jj32+p, c]/16
    """
    s = np.float16(1.0)  # applied after f32 * (1/16)
    at = (np.asarray(input_a, np.float32) * np.float32(1 / 16)).astype(np.float16)
    bt = (np.asarray(input_b, np.float32) * np.float32(1 / 16)).astype(np.float16)
    at = at.transpose(0, 3, 1, 2)  # [B, C, H, W]
    bt = bt.transpose(0, 3, 1, 2)
    nb = at.shape[0]
    ap = at.reshape(nb, C, 6, PACK, 2, 32, 2).transpose(0, 1, 4, 2, 6, 3, 5)
    bp = bt.reshape(nb, C, 24, 2, 32, 2).transpose(0, 1, 5, 3, 2, 4)
    del s
    return (np.ascontiguousarray(ap).reshape(nb, C, 2, 6, 2, PACK * 32),
            np.ascontiguousarray(bp).reshape(nb, C, 2, 2, 24 * 32))


def build_program():
    nc = bacc.Bacc("TRN2", target_bir_lowering=False, debug=False)

    a_d = nc.dram_tensor("a_t", [B_PER, C, 2, 6, 2, PACK * 32], F16,
                         kind="ExternalInput")
    b_d = nc.dram_tensor("b_t", [B_PER, C, 2, 2, 24 * 32], F16,
                         kind="ExternalInput")
    o_d = nc.dram_tensor("out_raw", [B_PER, N_GROUPS, 2, 128, OUT_W], F16,
                         kind="ExternalOutput")

    with tile.TileContext(nc) as tc, ExitStack() as ctx:
        inp = ctx.enter_context(tc.tile_pool(name="inp", bufs=1))
        psum = ctx.enter_context(
            tc.tile_pool(name="psum", bufs=4, space=bass.MemorySpace.PSUM))
        stage = ctx.enter_context(tc.tile_pool(name="stage", bufs=8))

        a_sb = {}
        b_sb = {}

        # Inputs stream as fine-grained quarters in exact PE consumption
        # order, ALTERNATING between the SP and ACT rings: the 16-engine DMA
        # pool round-robins across active queues, so two queues double the
        # config (descriptor-gen) rate while each queue's internal order
        # stays monotone in need-time — CoreSim's independent-queue model
        # and the HW pool then agree on arrival order.
        qi = 0
        for b in range(B_PER):
            for par in (0, 1):
                for cc in (0, 1):
                    cs = slice(cc * 128, (cc + 1) * 128)
                    ta = inp.tile([128, 6, 2, PACK * 32], F16,
                                  tag=f"aq{b}{cc}_{par}")
                    nc.sync.dma_start(ta[:], a_d[b, cs, par])
                    qi += 1
                    a_sb[b, cc, par] = ta
                    for p in (0, 1):
                        tb = inp.tile([128, 24 * 32], F16,
                                      tag=f"bq{b}{cc}_{p}{par}")
                        nc.sync.dma_start(tb[:], b_d[b, cs, p, par])
                        qi += 1
                        b_sb[b, cc, p, par] = tb

        def lhs_ap(b, cc, par, pk, p):
            return a_sb[b, cc, par][:, pk, p, :]

        def rhs_ap(b, cc, p, par, lo, hi):
            return b_sb[b, cc, p, par][:, lo:hi]

        # One unit per (b, gi, p): both r-chunks land in one 2-bank PSUM
        # supertile, drained by ONE copy and ONE DMA.
        t = 0
        for b in range(B_PER):
            for gi, (pack, r_list) in enumerate(GROUPS):
                par = pack[0] % 2
                pk = (pack[0] // 2) // PACK
                chunks = chunk_rs(r_list)
                nch = len(chunks)
                nr = len(chunks[0])
                ncols = nr * 32
                tot = nch * ncols
                for p in (0, 1):
                    ps = psum.tile([128, 1024], F32, tag="ps")
                    for cc in (0, 1):
                        for ci, rs in enumerate(chunks):
                            r2lo = rs[0] // 2
                            rhs = rhs_ap(b, cc, p, par,
                                         r2lo * 32, (r2lo + nr) * 32)
                            nc.tensor.matmul(
                                ps[:, ci * 512: ci * 512 + ncols],
                                lhs_ap(b, cc, par, pk, p), rhs,
                                start=(cc == 0), stop=(cc == 1),
                            )
                    st = stage.tile([128, OUT_W], F16, tag="st")
                    ps3 = ps.rearrange("q (c n) -> q c n", c=2)
                    st3 = st[:, :tot].rearrange("q (c n) -> q c n", c=nch)
                    if t % 2 == 0:
                        nc.vector.tensor_copy(st3, ps3[:, :nch, :ncols])
                    else:
                        nc.scalar.copy(st3, ps3[:, :nch, :ncols])
                    # outs: Pool ring for the first 2/3, SP ring (free once
                    # the input stream ends) for the tail.
                    eng = nc.gpsimd if t < 32 else nc.sync
                    eng.dma_start(o_d[b, gi, p][:, :tot], st[:, :tot])
                    t += 1

    nc.compile()
    return nc


_NC_CACHE = None


def _get_program():
    global _NC_CACHE
    if _NC_CACHE is None:
        _NC_CACHE = build_program()
    return _NC_CACHE


def assemble_output(raw_all):
    """raw_all: [nb, N_GROUPS, 2, 128, OUT_W] fp16 (scale already baked)
    -> out [nb, H, W, D*D] f32."""
    nb = raw_all.shape[0]
    raw_all = np.asarray(raw_all, np.float32)
    # band tensor: [nb, H, 2(p), 32(j32), D(dy), 32(jj32)]
    band = np.zeros((nb, H, 2, 32, D, 32), np.float32)
    for gi, (pack, r_list) in enumerate(GROUPS):
        chunks = chunk_rs(r_list)
        nr = len(chunks[0])
        for ci, rs in enumerate(chunks):
            # [nb, 2p, 128, nr*32] -> [nb, 2p, 4k, 32j, nr, 32jj]
            blk = raw_all[:, gi, :, :, ci * nr * 32:(ci + 1) * nr * 32]
            blk = blk.reshape(nb, 2, PACK, 32, nr, 32)
            for k, i in enumerate(pack):
                for ridx, r in enumerate(rs):
                    dy = r - i
                    if abs(dy) > MD:
                        continue
                    dyi = (dy + MD) // 2
                    band[:, i, :, :, dyi, :] = blk[:, :, k, :, ridx, :]
    out = np.zeros((nb, H, W, D, D), np.float32)
    s = band.strides
    for p in (0, 1):
        for ti in range(D):
            delta = ti - MD // 2  # dx/2
            j32_lo = max(0, -delta)
            j32_hi = min(32, 32 - delta)
            n = j32_hi - j32_lo
            if n <= 0:
                continue
            v = np.lib.stride_tricks.as_strided(
                band[:, :, p, j32_lo:, :, j32_lo + delta:],
                shape=(nb, H, n, D),
                strides=(s[0], s[1], s[3] + s[5], s[4]),
            )
            out[:, :, 2 * np.arange(j32_lo, j32_hi) + p, :, ti] = \
                v.transpose(2, 0, 1, 3)
    return out.reshape(nb, H, W, D * D)


def kernel(input_a: np.ndarray, input_b: np.ndarray) -> np.ndarray:
    from concourse.bass_utils import run_bass_kernel_spmd

    a, bt = prep_inputs(input_a, input_b)
    nc = _get_program()
    core_ids = list(range(N_CORES))
    in_maps = [
        {"a_t": a[c * B_PER:(c + 1) * B_PER], "b_t": bt[c * B_PER:(c + 1) * B_PER]}
        for c in core_ids
    ]
    res = run_bass_kernel_spmd(nc, in_maps, core_ids)
    raw_all = np.concatenate(
        [res.results[c]["out_raw"] for c in core_ids], axis=0)
    return assemble_output(raw_all)


# revision 21
# speedup vs baseline: 1.0960x; 1.0184x over previous
"""Trainium2 Bass kernel for FlowNet-C CorrelationCost.

Problem: out[b,i,j, tj*21+ti] = (1/C) * sum_c A[b,i,j,c] * Bz[b, i+dy, j+dx, c]
with dy = 2*tj - 20, dx = 2*ti - 20, Bz = B zero-padded by 20 spatially.
Shapes: A, B = [16, 48, 64, 256] f32 -> out [16, 48, 64, 441] f32.

Strategy (v2)
-------------
- Pure data-parallel: batch 16 -> 2 images per NeuronCore (8 cores, SPMD).
- Host pre-transposes to channel-major, prescales by 1/16 (so the two input
  scales bake the exact 1/256 output scale) and rounds to fp16. Tolerance is
  2e-2; fp16 inputs give ~1.5e-4, so no hi/lo split -> only 2 K-passes of
  128 channels each (3x less PE work than the fp32-exact hi/lo scheme).
- PE formulation: stationary = A[c, 4 same-parity rows x 32 same-parity
  cols] (128x128); moving = B[c, r x 32 same-parity cols] for all B rows r
  within +-20 of the pack. PSUM[(i,j), (r,jj)] holds every correlation
  product with dy = r - i, dx = jj - j.
- Per (batch, group, col-parity): one 2-bank PSUM supertile holds both
  r-chunks; the 2 matmul passes accumulate fp32; ONE copy (alternating
  VectorE / ScalarE) casts PSUM -> fp16 SBUF; ONE DMA (alternating SP ring /
  Pool ring) ships the compact block. Output bytes are halved vs fp32.
- Input DMAs: batch 0 arrives as fine-grained quarters on the SP ring (PE
  unblocks ~2.5us in); batch 1 as big per-(b,chunk) tiles on the Pool ring.

The harness calls kernel(**inputs) with the FULL inputs; this file is
self-contained (shapes hardcoded).
"""

from contextlib import ExitStack

import numpy as np

import concourse.bass as bass
import concourse.tile as tile
from concourse import bacc, mybir

B_FULL, H, W, C = 16, 48, 64, 256
N_CORES = 8
B_PER = B_FULL // N_CORES  # batches per core
MD = 20                    # max displacement
D = 21                     # displacements per axis
PACK = 4                   # i rows packed into one stationary operand
F32 = mybir.dt.float32
F16 = mybir.dt.float16
N_CC = C // 128            # channel chunks (K-passes)
OUT_W = 704                # max per-(b,gi,p) out cols (2 chunks x 11 r x 32)


def plan_groups():
    """(pack, r_list) per i-pack: pack = 4 same-parity rows, r_list = B rows
    (same parity, step 2) needed by any row in the pack."""
    groups = []
    for par in (0, 1):
        i_vals = list(range(par, H, 2))
        for k in range(0, len(i_vals), PACK):
            pack = i_vals[k:k + PACK]
            r_lo = max(0, pack[0] - MD)
            r_hi = min(H - 1, pack[-1] + MD)
            r_list = [r for r in range(r_lo, r_hi + 1) if (r - pack[0]) % 2 == 0]
            groups.append((pack, r_list))
    return groups


def chunk_rs(r_list):
    """Split the r list into chunks of <= 16 rows (<= 512 cols, one PSUM
    bank)."""
    n = len(r_list)
    if n <= 16:
        return [r_list]
    h = (n + 1) // 2
    return [r_list[:h], r_list[h:]]


GROUPS = plan_groups()
N_GROUPS = len(GROUPS)  # 12 i-packs


def prep_inputs(input_a, input_b):
    """Full [B, H, W, C] f32 inputs -> packed fp16 device arrays.

    a_packed[b, c, par, pk, p, k*32+j32] = a[b, 8*pk+2*k+par, 2*j32+p, c]/16
    b_packed[b, c, p, par, r2*32+jj32]  = b[b, 2*r2+par, 2*jj32+p, c]/16
    """
    s = np.float16(1.0)  # applied after f32 * (1/16)
    at = (np.asarray(input_a, np.float32) * np.float32(1 / 16)).astype(np.float16)
    bt = (np.asarray(input_b, np.float32) * np.float32(1 / 16)).astype(np.float16)
    at = at.transpose(0, 3, 1, 2)  # [B, C, H, W]
    bt = bt.transpose(0, 3, 1, 2)
    nb = at.shape[0]
    ap = at.reshape(nb, C, 6, PACK, 2, 32, 2).transpose(0, 1, 4, 2, 6, 3, 5)
    bp = bt.reshape(nb, C, 24, 2, 32, 2).transpose(0, 1, 5, 3, 2, 4)
    del s
    return (np.ascontiguousarray(ap).reshape(nb, C, 2, 6, 2, PACK * 32),
            np.ascontiguousarray(bp).reshape(nb, C, 2, 2, 24 * 32))


def build_program():
    nc = bacc.Bacc("TRN2", target_bir_lowering=False, debug=False)

    a_d = nc.dram_tensor("a_t", [B_PER, C, 2, 6, 2, PACK * 32], F16,
                         kind="ExternalInput")
    b_d = nc.dram_tensor("b_t", [B_PER, C, 2, 2, 24 * 32], F16,
                         kind="ExternalInput")
    o_d = nc.dram_tensor("out_raw", [B_PER, N_GROUPS, 2, 128, OUT_W], F16,
                         kind="ExternalOutput")

    with tile.TileContext(nc) as tc, ExitStack() as ctx:
        inp = ctx.enter_context(tc.tile_pool(name="inp", bufs=1))
        psum = ctx.enter_context(
            tc.tile_pool(name="psum", bufs=3, space=bass.MemorySpace.PSUM))
        stage = ctx.enter_context(tc.tile_pool(name="stage", bufs=8))

        # Warmup / gap-filler: dummy matmuls on a scratch PSUM bank keep the
        # PE array continuously busy through input-paced stalls so the DVFS
        # p-state ramps to (and stays at) max instead of resetting to 1.2GHz.
        dm = inp.tile([128, 512], F16, tag="warm_src")
        nc.vector.memset(dm[:], 0.0)
        wps = psum.tile([128, 512], F32, tag="warm_ps", bufs=1)

        def warm(n):
            for _ in range(n):
                nc.tensor.matmul(wps[:, :512], dm[:, :128], dm[:, :512],
                                 start=True, stop=True)

        a_sb = {}
        b_sb = {}

        # Inputs stream as fine-grained quarters in exact PE consumption
        # order, ALTERNATING between the SP and ACT rings: the 16-engine DMA
        # pool round-robins across active queues, so two queues double the
        # config (descriptor-gen) rate while each queue's internal order
        # stays monotone in need-time — CoreSim's independent-queue model
        # and the HW pool then agree on arrival order.
        qi = 0
        for b in range(B_PER):
            for par in (0, 1):
                for cc in (0, 1):
                    cs = slice(cc * 128, (cc + 1) * 128)
                    ta = inp.tile([128, 6, 2, PACK * 32], F16,
                                  tag=f"aq{b}{cc}_{par}")
                    nc.sync.dma_start(ta[:], a_d[b, cs, par])
                    qi += 1
                    a_sb[b, cc, par] = ta
                    for p in (0, 1):
                        tb = inp.tile([128, 24 * 32], F16,
                                      tag=f"bq{b}{cc}_{p}{par}")
                        nc.sync.dma_start(tb[:], b_d[b, cs, p, par])
                        qi += 1
                        b_sb[b, cc, p, par] = tb

        def lhs_ap(b, cc, par, pk, p):
            return a_sb[b, cc, par][:, pk, p, :]

        def rhs_ap(b, cc, p, par, lo, hi):
            return b_sb[b, cc, p, par][:, lo:hi]

        # One unit per (b, gi, p): both r-chunks land in one 2-bank PSUM
        # supertile, drained by ONE copy and ONE DMA.
        warm(20)
        t = 0
        for b in range(B_PER):
            for gi, (pack, r_list) in enumerate(GROUPS):
                par = pack[0] % 2
                pk = (pack[0] // 2) // PACK
                chunks = chunk_rs(r_list)
                nch = len(chunks)
                nr = len(chunks[0])
                ncols = nr * 32
                tot = nch * ncols
                for p in (0, 1):
                    if t in (12, 24, 36):
                        warm({12: 8, 24: 6, 36: 4}[t])
                    ps = psum.tile([128, 1024], F32, tag="ps")
                    for cc in (0, 1):
                        for ci, rs in enumerate(chunks):
                            r2lo = rs[0] // 2
                            rhs = rhs_ap(b, cc, p, par,
                                         r2lo * 32, (r2lo + nr) * 32)
                            nc.tensor.matmul(
                                ps[:, ci * 512: ci * 512 + ncols],
                                lhs_ap(b, cc, par, pk, p), rhs,
                                start=(cc == 0), stop=(cc == 1),
                            )
                    st = stage.tile([128, OUT_W], F16, tag="st")
                    ps3 = ps.rearrange("q (c n) -> q c n", c=2)
                    st3 = st[:, :tot].rearrange("q (c n) -> q c n", c=nch)
                    if t % 2 == 0:
                        nc.vector.tensor_copy(st3, ps3[:, :nch, :ncols])
                    else:
                        nc.scalar.copy(st3, ps3[:, :nch, :ncols])
                    # outs: Pool ring for the first 2/3, SP ring (free once
                    # the input stream ends) for the tail.
                    eng = nc.gpsimd if t < 32 else nc.sync
                    eng.dma_start(o_d[b, gi, p][:, :tot], st[:, :tot])
                    t += 1

    nc.compile()
    return nc


_NC_CACHE = None


def _get_program():
    global _NC_CACHE
    if _NC_CACHE is None:
        _NC_CACHE = build_program()
    return _NC_CACHE


def assemble_output(raw_all):
    """raw_all: [nb, N_GROUPS, 2, 128, OUT_W] fp16 (scale already baked)
    -> out [nb, H, W, D*D] f32."""
    nb = raw_all.shape[0]
    raw_all = np.asarray(raw_all, np.float32)
    # band tensor: [nb, H, 2(p), 32(j32), D(dy), 32(jj32)]
    band = np.zeros((nb, H, 2, 32, D, 32), np.float32)
    for gi, (pack, r_list) in enumerate(GROUPS):
        chunks = chunk_rs(r_list)
        nr = len(chunks[0])
        for ci, rs in enumerate(chunks):
            # [nb, 2p, 128, nr*32] -> [nb, 2p, 4k, 32j, nr, 32jj]
            blk = raw_all[:, gi, :, :, ci * nr * 32:(ci + 1) * nr * 32]
            blk = blk.reshape(nb, 2, PACK, 32, nr, 32)
            for k, i in enumerate(pack):
                for ridx, r in enumerate(rs):
                    dy = r - i
                    if abs(dy) > MD:
                        continue
                    dyi = (dy + MD) // 2
                    band[:, i, :, :, dyi, :] = blk[:, :, k, :, ridx, :]
    out = np.zeros((nb, H, W, D, D), np.float32)
    s = band.strides
    for p in (0, 1):
        for ti in range(D):
            delta = ti - MD // 2  # dx/2
            j32_lo = max(0, -delta)
            j32_hi = min(32, 32 - delta)
            n = j32_hi - j32_lo
            if n <= 0:
                continue
            v = np.lib.stride_tricks.as_strided(
                band[:, :, p, j32_lo:, :, j32_lo + delta:],
                shape=(nb, H, n, D),
                strides=(s[0], s[1], s[3] + s[5], s[4]),
            )
            out[:, :, 2 * np.arange(j32_lo, j32_hi) + p, :, ti] = \
                v.transpose(2, 0, 1, 3)
    return out.reshape(nb, H, W, D * D)


def kernel(input_a: np.ndarray, input_b: np.ndarray) -> np.ndarray:
    from concourse.bass_utils import run_bass_kernel_spmd

    a, bt = prep_inputs(input_a, input_b)
    nc = _get_program()
    core_ids = list(range(N_CORES))
    in_maps = [
        {"a_t": a[c * B_PER:(c + 1) * B_PER], "b_t": bt[c * B_PER:(c + 1) * B_PER]}
        for c in core_ids
    ]
    res = run_bass_kernel_spmd(nc, in_maps, core_ids)
    raw_all = np.concatenate(
        [res.results[c]["out_raw"] for c in core_ids], axis=0)
    return assemble_output(raw_all)
